# revision 22
# baseline (speedup 1.0000x reference)
"""Position-attention kernel for Trainium2 (8 NeuronCores, SPMD data-parallel).

Math (per batch b):
    q = X Wq ; k = X Wk ; v = X Wv          (X = x[b] reshaped [N, C], N=4096)
    energy[i, j] = k_i . q_j ;  attn = softmax(energy, -1)
    out = gamma * (attn @ v) + X

v2 design (exp-bound pipeline):
  - Host precomputes qT/kT (rank-16 factors of the energy), gamma-folded V
    with a ones-column (softmax denominator for free), and the fp32 residual.
  - Energy eT[j,i] = qT^T kT via 32x32 tile-packed matmuls (tile_position):
    contraction is 17 wide (16 channels + a shift row baking in the -SHIFT
    softmax bias), so a 12-MM pack covers [384 j, 512 i] at ~4x PE efficiency.
  - exp on the scalar engine for most j-groups (FD=1536 activations); selected
    groups use a 1-op DVE Schraudolph exp: uint16(e*C1 + C2) bit-cast to bf16
    (saturating convert clamps the e^-90 underflow tail to +0).
  - attn@v: pt stationary [j, i-128], moving [v|1] (129 cols); one PSUM
    accumulation pass per 128-row output slice, two banks round-robin.
  - blend: reciprocal + scale on DVE, residual add on GPSIMD, out DMA on sync.

Sharding: 8 cores = (4 batches) x (2 halves of the 4096 output rows).
"""

import numpy as np

B, Dd, Hh, Ww, C = 4, 16, 16, 16, 128
N = Dd * Hh * Ww            # 4096 sequence positions (j)
NCORES = 8
NI = (B * N) // NCORES      # 2048 output rows per core (i)
NJB = N // 128              # 32 j-blocks
NTI = NI // 128             # 16 output row-slices (attn@v passes)
SHIFT = 32.0                # softmax shift (cancels in normalization)

# exp-group geometry: et tiles [128, 2, 512] (FD=1024, 2 PSUM banks) with 3
# pool bufs -> pipeline degree 3 over the exp->energy WAR chain; energy is
# emitted as paired 16-MM packs (4 row-grps x 4 col-grps) filling 2 tiles.
ETG = [(2 * g, 2) for g in range(NJB // 2)]
NG = len(ETG)               # 16 groups per energy i-chunk
NEIC = NI // 512            # 4 energy i-chunks

DVE_G = (1, 3, 5, 8, 10, 13, 15)   # groups per e-chunk exp'd on DVE
CCORR = 8.0
C1 = 128.0 / float(np.log(2.0))
C2 = 127.0 * 128.0 - CCORR
QUOTA = 24                  # attn@v matmuls emitted per pipeline step

_NC_CACHE = {}


def _build_nc():
    from contextlib import ExitStack

    import concourse.bacc as bacc
    import concourse.mybir as mybir
    import concourse.tile as tile

    dt = mybir.dt
    nc = bacc.Bacc(target_bir_lowering=False)

    qt_d = nc.declare_dram_parameter("qt", [128, N], dt.float16, isOutput=False)
    kt_d = nc.declare_dram_parameter("kt", [128, NI], dt.float16, isOutput=False)
    v_d = nc.declare_dram_parameter("v", [8, 128, 4, 132], dt.bfloat16, isOutput=False)
    xres_d = nc.declare_dram_parameter("xres", [NTI, 128, 128], dt.float32, isOutput=False)
    out_d = nc.declare_dram_parameter("out", [NTI, 128, 128], dt.float32, isOutput=True)

    with tile.TileContext(nc) as tc, ExitStack() as ctx:
        persist = ctx.enter_context(tc.tile_pool(name="persist", bufs=1))

        # warm the exp table while DMAs run
        dummy = persist.tile([1, 1], dt.float32)
        nc.vector.memset(dummy[:], 0.0)
        nc.scalar.activation(
            out=dummy[:], in_=dummy[:], func=mybir.ActivationFunctionType.Exp
        )
        warm = persist.tile([128, 64], dt.float16)
        nc.vector.memset(warm[:], 0.0)

        # tiny queue warmers
        qw = persist.tile([1, 4], dt.float16)
        nc.sync.dma_start(out=qw[0:1, 0:2], in_=qt_d[0:1, 0:2])
        nc.gpsimd.dma_start(out=qw[0:1, 2:4], in_=qt_d[0:1, 2:4])

        # per-chunk tiles so Tile's dependency tracking stays DMA-granular
        qt_ch = [persist.tile([128, 512], dt.float16, name=f"qt{m}") for m in range(8)]
        kt_ch = [persist.tile([128, 512], dt.float16, name=f"kt{e}") for e in range(4)]
        v_ch = [
            persist.tile([128, 4, 132], dt.bfloat16, name=f"v{jc}") for jc in range(8)
        ]

        # DMA order mirrors need-by times; scalar/vector/tensor queues carry
        # no input DMA.
        for idx, (eng, t, s) in enumerate(
            [
                (nc.sync, kt_ch[0], kt_d[:, 0:512]),
                (nc.gpsimd, qt_ch[0], qt_d[:, 0:512]),
                (nc.sync, qt_ch[1], qt_d[:, 512:1024]),
                (nc.gpsimd, v_ch[0], v_d[0]),
                (nc.sync, qt_ch[2], qt_d[:, 1024:1536]),
                (nc.gpsimd, v_ch[1], v_d[1]),
                (nc.sync, qt_ch[4], qt_d[:, 2048:2560]),
                (nc.gpsimd, qt_ch[3], qt_d[:, 1536:2048]),
                (nc.sync, v_ch[2], v_d[2]),
                (nc.gpsimd, kt_ch[1], kt_d[:, 512:1024]),
                (nc.sync, qt_ch[6], qt_d[:, 3072:3584]),
                (nc.gpsimd, qt_ch[5], qt_d[:, 2560:3072]),
                (nc.sync, v_ch[3], v_d[3]),
                (nc.gpsimd, v_ch[4], v_d[4]),
                (nc.sync, v_ch[6], v_d[6]),
                (nc.gpsimd, qt_ch[7], qt_d[:, 3584:4096]),
                (nc.sync, kt_ch[2], kt_d[:, 1024:1536]),
                (nc.gpsimd, v_ch[5], v_d[5]),
                (nc.sync, v_ch[7], v_d[7]),
                (nc.gpsimd, kt_ch[3], kt_d[:, 1536:2048]),
            ]
        ):
            eng.dma_start(out=t[:], in_=s)

        epool = ctx.enter_context(tc.tile_pool(name="ep", bufs=3, space="PSUM"))
        opool = ctx.enter_context(tc.tile_pool(name="op", bufs=1, space="PSUM"))
        ptpool = ctx.enter_context(tc.tile_pool(name="ptp", bufs=32))
        spool = ctx.enter_context(tc.tile_pool(name="sp", bufs=8))
        otpool = ctx.enter_context(tc.tile_pool(name="otp", bufs=4))
        xrpool = ctx.enter_context(tc.tile_pool(name="xrp", bufs=4))

        # PE warmup into the oaA psum slot (released before first real use)
        wt = opool.tile([128, 129], dt.float32, tag="oaA", name="warmp")
        for r in range(8):
            nc.tensor.matmul(wt[0:64, 0:64], warm[:], warm[:], start=True, stop=True)

        flat = [(k, g) for k in range(NEIC) for g in range(NG)]
        ets, pts = {}, {}

        def emit_energy_pack(eic, m):
            """16-MM pack filling two et tiles: groups (2m, 2m+1) = jb 4m..4m+3,
            one row-group of the array per jb."""
            ga, gb = 2 * m, 2 * m + 1
            eta = epool.tile([128, 2, 512], dt.float32, tag="et", name=f"et{eic}_{ga}")
            etb = epool.tile([128, 2, 512], dt.float32, tag="et", name=f"et{eic}_{gb}")
            for r in range(4):
                jb = 4 * m + r
                et = eta if r < 2 else etb
                for c4 in range(4):
                    nc.tensor.matmul(
                        et[32 * c4 : 32 * c4 + 32, r % 2, :],
                        qt_ch[jb // 4][
                            32 * r : 32 * r + 32,
                            (jb % 4) * 128 + 32 * c4 : (jb % 4) * 128 + 32 * c4 + 32,
                        ],
                        kt_ch[eic][32 * r : 32 * r + 32, :],
                        start=True,
                        stop=True,
                        tile_position=(32 * r, 32 * c4),
                    )
            ets[(eic, ga)] = eta
            ets[(eic, gb)] = etb

        def emit_exp(eic, g):
            et = ets.pop((eic, g))
            pt = ptpool.tile([128, 2, 512], dt.uint16, tag="pt", name=f"pt{eic}_{g}")
            if g in DVE_G:
                nc.vector.tensor_scalar(
                    out=pt[:],
                    in0=et[:],
                    scalar1=float(C1),
                    scalar2=float(C2),
                    op0=mybir.AluOpType.mult,
                    op1=mybir.AluOpType.add,
                )
            else:
                nc.scalar.activation(
                    out=pt.bitcast(dt.bfloat16)[:],
                    in_=et[:],
                    func=mybir.ActivationFunctionType.Exp,
                )
            pts[(eic, g)] = pt

        # --- attn@v chunk state (chunk c = output rows [256c, 256c+256)) ---
        # Odd chunks run 8 exp-groups behind so their blends (DVE work) land
        # mid-e-chunk instead of colliding with the next e-chunk's ramp-up.
        # Bank mates chosen so both last chunks can start promptly.
        NOA = NI // 256
        OA_TAG = {0: "oaA", 1: "oaB", 2: "oaA", 3: "oaB",
                  4: "oaA", 5: "oaB", 6: "oaB", 7: "oaA"}
        OA_MATE = {2: 0, 3: 1, 4: 2, 5: 3, 6: 5, 7: 4}
        OA_LAG = {1: 8, 3: 8, 5: 8}
        oa_t, xr_t = {}, {}
        jb_done = [0] * NOA          # next jb to emit for chunk c
        blended = [False] * NOA

        def start_chunk(c):
            # all-acc_flags=0 accumulation onto a DVE-zeroed bank: correct
            # whether or not has_written bits are set, and lets two
            # accumulation groups share one bank.
            oa = opool.tile([128, 2, 129], dt.float32, tag=OA_TAG[c], name=f"oa{c}")
            nc.vector.memset(oa[:], 0.0)
            oa_t[c] = oa
            xr = xrpool.tile([128, 2, 128], dt.float32, tag="xr", name=f"xr{c}")
            nc.gpsimd.dma_start(out=xr[:, 0, :], in_=xres_d[2 * c])
            nc.gpsimd.dma_start(out=xr[:, 1, :], in_=xres_d[2 * c + 1])
            xr_t[c] = xr

        def emit_attnv(c, jb):
            eic = c // 2
            g, gg = jb // 2, jb % 2
            pt_bf = pts[(eic, g)].bitcast(mybir.dt.bfloat16)
            for it in range(2):
                ioff = (c % 2) * 256 + it * 128
                nc.tensor.matmul(
                    oa_t[c][:, it, :],
                    pt_bf[:, gg, ioff : ioff + 128],
                    v_ch[jb // 4][:, jb % 4, 0:129],
                    start=False,
                    stop=(jb == NJB - 1),
                    skip_group_check=True,
                )

        def emit_blend(c):
            oa = oa_t.pop(c)
            xr = xr_t.pop(c)
            late = c >= 6          # after the last exp: vector + scalar are free
            rs = spool.tile([128, 2], dt.float32, tag="rs", name=f"rs{c}")
            nc.vector.reciprocal(rs[:], oa[:, :, 128:129])
            for it in range(2):
                ot = otpool.tile([128, 128], dt.float32, tag="ot", name=f"ot{c}_{it}")
                nc.vector.tensor_scalar(
                    out=ot[:],
                    in0=oa[:, it, 0:128],
                    scalar1=rs[:, it : it + 1],
                    scalar2=None,
                    op0=mybir.AluOpType.mult,
                )
                tt_eng = nc.vector if late else nc.gpsimd
                tt_eng.tensor_tensor(
                    out=ot[:], in0=ot[:], in1=xr[:, it, :], op=mybir.AluOpType.add
                )
                dma_eng = (nc.scalar if it else nc.sync) if late else nc.sync
                dma_eng.dma_start(out=out_d[2 * c + it], in_=ot[:])
            blended[c] = True

        def drain_attnv(fk, quota):
            """Greedy: emit ready attn@v MMs (2 per jb), lowest chunk first.
            A chunk may only start once its bank-mate (c-2) is blended; group g
            is ready strictly after its exp step (PE-FIFO safety)."""
            for c in range(NOA):
                if quota <= 0:
                    return
                if blended[c]:
                    continue
                if c in OA_MATE and not blended[OA_MATE[c]]:
                    continue
                gready = min(NG, fk - (c // 2) * NG - OA_LAG.get(c, 0))
                if gready <= 0:
                    continue
                jb_ready = ETG[gready - 1][0] + ETG[gready - 1][1]
                if c not in oa_t and jb_done[c] < jb_ready:
                    start_chunk(c)
                while jb_done[c] < jb_ready and quota > 0:
                    emit_attnv(c, jb_done[c])
                    jb_done[c] += 1
                    quota -= 2
                if jb_done[c] == NJB:
                    emit_blend(c)

        emit_energy_pack(0, 0)
        emit_energy_pack(0, 1)
        for fk, (eic, g) in enumerate(flat):
            for ahead in (2, 3):
                if fk + ahead < len(flat) and flat[fk + ahead] not in ets:
                    ne, ng = flat[fk + ahead]
                    emit_energy_pack(ne, ng // 2)
            drain_attnv(fk, QUOTA)
            emit_exp(eic, g)
        while not all(blended):
            drain_attnv(len(flat), QUOTA)

    nc.finalize()
    return nc


def get_nc():
    if "nc" not in _NC_CACHE:
        _NC_CACHE["nc"] = _build_nc()
    return _NC_CACHE["nc"]


def _to_bf16(a):
    import ml_dtypes

    return a.astype(ml_dtypes.bfloat16)


def make_in_maps(x, Wq, Wk, Wv, gamma):
    x = np.asarray(x, dtype=np.float64)
    Wq = np.asarray(Wq, dtype=np.float64)
    Wk = np.asarray(Wk, dtype=np.float64)
    Wv = np.asarray(Wv, dtype=np.float64)
    gamma = float(np.asarray(gamma).reshape(-1)[0])

    xf = x.reshape(B, N, C)
    in_maps = []
    for c in range(NCORES):
        b, ih = c // 2, c % 2
        q = xf[b] @ Wq                      # [N, 16]
        k = xf[b] @ Wk                      # [N, 16]
        v = gamma * (xf[b] @ Wv)            # [N, C], gamma folded in
        isl = slice(ih * NI, (ih + 1) * NI)

        qt = np.zeros((128, N), dtype=np.float16)
        ktr = np.zeros((128, NI), dtype=np.float16)
        for r in range(4):
            qt[32 * r : 32 * r + 16] = q.T.astype(np.float16)
            qt[32 * r + 16] = 1.0
            ktr[32 * r : 32 * r + 16] = k[isl].T.astype(np.float16)
            ktr[32 * r + 16] = -SHIFT
        vv = np.zeros((8, 128, 4, 132), dtype=np.float32)
        vr = v.reshape(8, 4, 128, C)
        for jc in range(8):
            for kk in range(4):
                vv[jc, :, kk, 0:128] = vr[jc, kk]
                vv[jc, :, kk, 128] = 1.0
        in_maps.append(
            {
                "qt": np.ascontiguousarray(qt),
                "kt": np.ascontiguousarray(ktr),
                "v": _to_bf16(vv),
                "xres": np.ascontiguousarray(
                    xf[b][isl].reshape(NTI, 128, 128).astype(np.float32)
                ),
            }
        )
    return in_maps


def assemble_out(results):
    outs = [np.asarray(results[c]["out"]).reshape(NI, C) for c in range(NCORES)]
    full = np.stack(
        [np.concatenate([outs[2 * b], outs[2 * b + 1]], axis=0) for b in range(B)]
    )
    return full.reshape(B, Dd, Hh, Ww, C).astype(np.float32)


def kernel(x, Wq, Wk, Wv, gamma):
    from concourse.bass_utils import run_bass_kernel_spmd

    nc = get_nc()
    in_maps = make_in_maps(x, Wq, Wk, Wv, gamma)
    res = run_bass_kernel_spmd(nc, in_maps, core_ids=list(range(NCORES)))
    return assemble_out(res.results)
